# revision 17
# baseline (speedup 1.0000x reference)
"""3D Haar DWT (clean-mode subband stack) on 8 Trainium2 NeuronCores, bf16 I/O.

Problem (hardcoded): inputs (4, 128, 128, 128, 4) f32, A (128, 128) f32 Haar
analysis operator. Output (4, 64, 64, 64, 32) f32 = 8 subbands stacked on the
channel axis (LLL, LLH, LHL, LHH, HLL, HLH, HHL, HHH) x 4 channels.

Sharding: pure data parallel over (batch, d1-half): core k handles
b = k // 2, d1 range [64*(k%2), 64*(k%2)+64). The Haar transform is a 2-tap
non-overlapping filter (rows of A touch only columns 2i, 2i+1), so splitting
d1 on an even boundary requires no communication.

The rel-err gate is 2e-2; a full bf16 pipeline measures ~6.5e-3, so all
device I/O is bf16 — 16 MiB/core of HBM traffic instead of 32, putting the
DMA roofline at ~47 us instead of ~94.

Per-core pipeline (slab host-converted to bf16 [d2, d1, d3par, d3pair, c],
i.e. even/odd d3 de-interleaved so every engine AP is contiguous):
  1. DMA in 1 MiB chunks (8 d1 slices), partitions = d2, 8 KiB runs.
     First/last 8 d1 are split 4+4 to shorten pipeline fill/drain.
  2. d3 butterfly on DVE (2 ops/chunk, all-contiguous bf16, 2x mode).
  3. d1 butterfly: sub on DVE; add offloaded to GPSIMD on the big middle
     chunks (it has slack after the store consolidation).
  4. d2 transform as one PE matmul per (s1, o1) slice with a single
     stationary weight matrix bf16(0.5*A^T) — both s2 halves come out on
     the PSUM partition axis, so each input column streams through PE once.
  5. PSUM -> SBUF evacuation on ACT: a single fully-contiguous 2048-elem
     copy/convert per 4 PSUM banks.
  6. One SWDGE store per chunk; y laid out [i2, o1, s1, s3, mc] so each
     store is one 8 KiB contiguous run per partition.

Scale bookkeeping: reference applies A (entries +-s, s=1/sqrt(2)) once per
axis: total s^3 per path. Here the d3/d1 butterflies apply +-1 and the
matmul applies 0.5*A (one s), so each path gets 0.5*s = s^3 exactly.
"""

import sys

import numpy as np

if "/opt/trn_rl_repo" not in sys.path:
    sys.path.insert(0, "/opt/trn_rl_repo")

B, N, C = 4, 128, 4
N_CORES = 8
SLAB = 64          # d1 extent per core
# (d1_start, d1_count) chunks: small ends shorten pipeline fill/drain
CHUNKS = (
    [(0, 4), (4, 4)]
    + [(8 + 12 * i, 12) for i in range(4)]
    + [(56, 4), (60, 4)]
)
GPSIMD_D1ADD = False  # gpsimd tensor ops are 4x slower and sat on the
                      # critical path (v2 measured +5.5 us); keep it DMA-only

_BASS_CACHE = {}


def _haar_matrix():
    s = np.float32(1.0 / np.sqrt(2.0))
    A = np.zeros((N, N), dtype=np.float32)
    for i in range(N // 2):
        A[i, 2 * i] = s
        A[i, 2 * i + 1] = s
        A[64 + i, 2 * i] = -s
        A[64 + i, 2 * i + 1] = s
    return A


def _reference_numpy(inputs, A):
    # Fallback only: exact reference math on host (used if A is not Haar).
    x = np.einsum("ij,bpjqc->bpiqc", A, inputs)
    x = np.einsum("ij,bjpqc->bipqc", A, x)
    x = np.einsum("ij,bpqjc->bpqic", A, x)
    m = x.shape[1] // 2
    subs = [
        x[:, :m, :m, :m, :], x[:, :m, :m, m:, :],
        x[:, :m, m:, :m, :], x[:, :m, m:, m:, :],
        x[:, m:, :m, :m, :], x[:, m:, :m, m:, :],
        x[:, m:, m:, :m, :], x[:, m:, m:, m:, :],
    ]
    return np.concatenate(subs, axis=-1).astype(np.float32)


def _build_bass():
    import concourse.bacc as bacc
    import concourse.mybir as mybir
    import concourse.tile as tile

    f32 = mybir.dt.float32
    bf16 = mybir.dt.bfloat16

    # Bacc (not raw Bass): its compile() pipeline splits multi-sem waits into
    # EventSemaphore instructions — TRN2 instructions have one wait slot.
    nc = bacc.Bacc("TRN2", target_bir_lowering=False, debug=False)
    # x is host-pre-arranged to [d2, d1, t, m, c] (d3 = 2m + t) so the d3
    # butterfly reads/writes contiguous 512-elem runs per d1 slice.
    x = nc.dram_tensor("x", [N, SLAB, 2, 64, C], bf16, kind="ExternalInput")
    atf = nc.dram_tensor("atf", [N, N], bf16, kind="ExternalInput")
    atn = nc.dram_tensor("atn", [N, N], bf16, kind="ExternalInput")
    # y dims: (i2, o1, s1, s3, m*c) with i2 = s2*64 + o2 on the partition
    # axis. A chunk store is one 8 KiB contiguous run per partition.
    y = nc.dram_tensor("y", [N, 32, 2, 2, 64 * C], bf16, kind="ExternalOutput")

    with tile.TileContext(nc) as tc:
        with (
            tc.tile_pool(name="const", bufs=1) as cpool,
            tc.tile_pool(name="io", bufs=3) as tpool,
            tc.tile_pool(name="mid", bufs=3) as mpool,
            tc.tile_pool(name="out", bufs=3) as opool,
            tc.tile_pool(name="psum", bufs=2, space="PSUM") as ppool,
        ):
            atf_sb = cpool.tile([N, N], bf16)
            atn_sb = cpool.tile([N, N], bf16)

            for ki, (st, cnt) in enumerate(CHUNKS):
                # 1. load chunk: one DMA, 128 runs of cnt KiB.
                T = tpool.tile([N, cnt, 2, 64 * C], bf16, tag="T")
                nc.sync.dma_start(
                    out=T[:],
                    in_=x[:, st:st + cnt].rearrange("p a t m c -> p a t (m c)"),
                )
                if ki == 0:
                    # weights after the first bulk load is issued so the data
                    # pipeline starts immediately
                    nc.sync.dma_start(out=atf_sb[:], in_=atf[:, :])
                    nc.sync.dma_start(out=atn_sb[:], in_=atn[:, :])

                # Part of each chunk skips the DVE d3 butterfly; PE folds
                # it into PSUM accumulation instead (PE has slack). 4-wide
                # chunks fold entirely, shortening the DVE fill/drain chain.
                pe_cnt = 4 if cnt <= 4 else (4 if cnt == 12 else 0)
                dv_cnt = cnt - pe_cnt

                # 2. d3 butterfly (contiguous): W[:,:,0] = even+odd, [:,:,1] = odd-even
                # 3. d1 butterfly: U[:, 0, g] = W(2g)+W(2g+1), U[:, 1, g] = diff
                # (for the PE set, the same butterfly runs on raw T slices)
                U = mpool.tile([N, 2, cnt // 2, 512], bf16, tag="U")
                gd = dv_cnt // 2
                if dv_cnt:
                    W = mpool.tile([N, dv_cnt, 2, 64 * C], bf16, tag="W")
                    nc.vector.tensor_add(
                        out=W[:, :, 0], in0=T[:, :dv_cnt, 0], in1=T[:, :dv_cnt, 1]
                    )
                    nc.vector.tensor_sub(
                        out=W[:, :, 1], in0=T[:, :dv_cnt, 1], in1=T[:, :dv_cnt, 0]
                    )
                    Wp = W[:].rearrange("p (g u) t f -> p g u (t f)", u=2)
                    nc.vector.tensor_add(
                        out=U[:, 0, :gd], in0=Wp[:, :, 0], in1=Wp[:, :, 1]
                    )
                    nc.vector.tensor_sub(
                        out=U[:, 1, :gd], in0=Wp[:, :, 1], in1=Wp[:, :, 0]
                    )
                if pe_cnt:
                    Tp = T[:, dv_cnt:].rearrange("p (g u) t f -> p g u (t f)", u=2)
                    nc.vector.tensor_add(
                        out=U[:, 0, gd:], in0=Tp[:, :, 0], in1=Tp[:, :, 1]
                    )
                    nc.vector.tensor_sub(
                        out=U[:, 1, gd:], in0=Tp[:, :, 1], in1=Tp[:, :, 0]
                    )

                # staging: (o1_loc, s1, s3*m*c) — matches the y layout
                Yst = opool.tile([N, cnt // 2, 2, 512], bf16, tag="Yst")

                n_o1 = cnt // 2
                for q in range((n_o1 + 1) // 2):
                    # 4. d2 transform. DVE-set banks: one 512-row matmul per
                    # (o1, s1) slice (rhs already d3-butterflied). PE-set
                    # banks: the d3 butterfly rides the PSUM accumulation as
                    # two 256-row passes per half with +-0.5*A^T stationary.
                    jj = min(2, n_o1 - 2 * q)  # o1 slices in this PSUM group
                    ps = ppool.tile([N, jj, 2, 512], f32, tag="ps")
                    if 2 * q >= gd:
                        Uv = U[:].rearrange("p s g (u f) -> p s g u f", u=2)
                        # atp passes first, atn passes last: 2 ldweights/group
                        for j in range(jj):
                            for s1 in range(2):
                                g = 2 * q + j
                                bank = ps[:, j, s1].rearrange(
                                    "p (k f) -> p k f", k=2
                                )
                                nc.tensor.matmul(
                                    bank[:, 0], lhsT=atf_sb[:],
                                    rhs=Uv[:, s1, g, 0],
                                    start=True, stop=False,
                                )
                                nc.tensor.matmul(
                                    bank[:, 0], lhsT=atf_sb[:],
                                    rhs=Uv[:, s1, g, 1],
                                    start=False, stop=True,
                                )
                                nc.tensor.matmul(
                                    bank[:, 1], lhsT=atf_sb[:],
                                    rhs=Uv[:, s1, g, 1],
                                    start=True, stop=False,
                                )
                        for j in range(jj):
                            for s1 in range(2):
                                g = 2 * q + j
                                bank = ps[:, j, s1].rearrange(
                                    "p (k f) -> p k f", k=2
                                )
                                nc.tensor.matmul(
                                    bank[:, 1], lhsT=atn_sb[:],
                                    rhs=Uv[:, s1, g, 0],
                                    start=False, stop=True,
                                )
                    else:
                        for j in range(jj):
                            for s1 in range(2):
                                nc.tensor.matmul(
                                    ps[:, j, s1], lhsT=atf_sb[:],
                                    rhs=U[:, s1, 2 * q + j],
                                    start=True, stop=True,
                                )
                    # 5. evacuate: fully-contiguous 2048-elem copy f32->bf16
                    # (ACT; measured faster than a DVE CAST and its queue is
                    # already clear when the final chunk's matmuls finish)
                    nc.scalar.copy(Yst[:, 2 * q:2 * q + jj], ps[:])

                # 6. one store per chunk on SWDGE (gpsimd): stores never
                # head-of-line-block the load queue, and chunk-sized stores
                # keep 8-12 KiB contiguous runs per partition (DMA engines
                # lose ~25% per-packet efficiency at 4 KiB).
                o1s = st // 2
                nc.gpsimd.dma_start(
                    out=y[:, o1s:o1s + cnt // 2].rearrange(
                        "p q a k f -> p (q a k f)"
                    ),
                    in_=Yst[:].rearrange("p q a f -> p (q a f)"),
                )

    # All matmuls share one stationary matrix, but tile legalization emits an
    # InstLdweights per matmul (~130 ns of PE each). The PE array retains the
    # stationary between matmuls, so drop redundant loads, keeping one per
    # 4-matmul group: Bacc's move_matmul_waits_to_ldweights later merges a
    # matmul's extra waits onto the most recent ldweights, and per-group
    # retention keeps that merge target in its original program position.
    for blk in nc.main_func.blocks:
        keep = []
        mm_since_kept = 0
        last_key = None
        for i in blk.instructions:
            if isinstance(i, mybir.InstMatmult):
                mm_since_kept += 1
            elif isinstance(i, mybir.InstLdweights):
                si = i.sync_info
                clean = not si or (len(si.on_wait) == 0 and len(si.on_update) == 0)
                key = (i.ins[0].memref, i.ins[0].offset)
                if clean and key == last_key and mm_since_kept < 4:
                    continue
                last_key = key
                mm_since_kept = 0
            keep.append(i)
        blk.instructions[:] = keep

    nc.compile()
    return nc


def _make_in_maps(x, A):
    import ml_dtypes

    atf = np.ascontiguousarray(0.5 * A.T).astype(ml_dtypes.bfloat16)
    atn = np.ascontiguousarray(-0.5 * A.T).astype(ml_dtypes.bfloat16)
    in_maps = []
    for k in range(N_CORES):
        b, h = divmod(k, 2)
        # pre-arrange slab to [d2, d1, t, m, c] (d3 de-interleaved)
        xs = (
            x[b, h * SLAB:(h + 1) * SLAB]
            .reshape(SLAB, N, 64, 2, C)
            .transpose(1, 0, 3, 2, 4)
        )
        in_maps.append(
            {
                "x": np.ascontiguousarray(xs).astype(ml_dtypes.bfloat16),
                "atf": atf,
                "atn": atn,
            }
        )
    return in_maps


def kernel(**inputs):
    x = np.asarray(inputs["inputs"], dtype=np.float32)
    A = np.asarray(inputs["A"], dtype=np.float32)
    assert x.shape == (B, N, N, N, C), x.shape

    if not np.allclose(A, _haar_matrix(), atol=1e-5):
        # Kernel hardcodes the 2-tap Haar structure; fall back for generic A.
        return _reference_numpy(x, A)

    from concourse.bass_utils import run_bass_kernel_spmd

    if "nc" not in _BASS_CACHE:
        _BASS_CACHE["nc"] = _build_bass()
    nc = _BASS_CACHE["nc"]

    res = run_bass_kernel_spmd(
        nc, _make_in_maps(x, A), core_ids=list(range(N_CORES))
    )

    out = np.empty((B, 64, 64, 64, 8 * C), np.float32)
    for k in range(N_CORES):
        b, h = divmod(k, 2)
        # y: [i2, o1, s1, s3, m, c] with i2 = s2*64 + o2, o1 local to slab
        arr = np.asarray(res.results[k]["y"], dtype=np.float32).reshape(
            2, 64, 32, 2, 2, 64, C
        )
        # (s2, o2, o1, s1, s3, m, c) -> (o1, o2, m, s1, s2, s3, c)
        out[b, 32 * h:32 * h + 32] = (
            arr.transpose(2, 1, 5, 3, 0, 4, 6).reshape(32, 64, 64, 8 * C)
        )
    return out


# revision 18
# speedup vs baseline: 1.0077x; 1.0077x over previous
"""3D Haar DWT (clean-mode subband stack) on 8 Trainium2 NeuronCores, bf16 I/O.

Problem (hardcoded): inputs (4, 128, 128, 128, 4) f32, A (128, 128) f32 Haar
analysis operator. Output (4, 64, 64, 64, 32) f32 = 8 subbands stacked on the
channel axis (LLL, LLH, LHL, LHH, HLL, HLH, HHL, HHH) x 4 channels.

Sharding: pure data parallel over (batch, d1-half): core k handles
b = k // 2, d1 range [64*(k%2), 64*(k%2)+64). The Haar transform is a 2-tap
non-overlapping filter (rows of A touch only columns 2i, 2i+1), so splitting
d1 on an even boundary requires no communication.

The rel-err gate is 2e-2; a full bf16 pipeline measures ~6.5e-3, so all
device I/O is bf16 — 16 MiB/core of HBM traffic instead of 32, putting the
DMA roofline at ~47 us instead of ~94.

Per-core pipeline (slab host-converted to bf16 [d2, d1, d3par, d3pair, c],
i.e. even/odd d3 de-interleaved so every engine AP is contiguous):
  1. DMA in 1 MiB chunks (8 d1 slices), partitions = d2, 8 KiB runs.
     First/last 8 d1 are split 4+4 to shorten pipeline fill/drain.
  2. d3 butterfly on DVE (2 ops/chunk, all-contiguous bf16, 2x mode).
  3. d1 butterfly: sub on DVE; add offloaded to GPSIMD on the big middle
     chunks (it has slack after the store consolidation).
  4. d2 transform as one PE matmul per (s1, o1) slice with a single
     stationary weight matrix bf16(0.5*A^T) — both s2 halves come out on
     the PSUM partition axis, so each input column streams through PE once.
  5. PSUM -> SBUF evacuation on ACT: a single fully-contiguous 2048-elem
     copy/convert per 4 PSUM banks.
  6. One SWDGE store per chunk; y laid out [i2, o1, s1, s3, mc] so each
     store is one 8 KiB contiguous run per partition.

Scale bookkeeping: reference applies A (entries +-s, s=1/sqrt(2)) once per
axis: total s^3 per path. Here the d3/d1 butterflies apply +-1 and the
matmul applies 0.5*A (one s), so each path gets 0.5*s = s^3 exactly.
"""

import sys

import numpy as np

if "/opt/trn_rl_repo" not in sys.path:
    sys.path.insert(0, "/opt/trn_rl_repo")

B, N, C = 4, 128, 4
N_CORES = 8
SLAB = 64          # d1 extent per core
# (d1_start, d1_count) chunks: small ends shorten pipeline fill/drain
CHUNKS = (
    [(0, 4), (4, 4)]
    + [(8 + 12 * i, 12) for i in range(4)]
    + [(56, 4), (60, 4)]
)
GPSIMD_D1ADD = False  # gpsimd tensor ops are 4x slower and sat on the
                      # critical path (v2 measured +5.5 us); keep it DMA-only

_BASS_CACHE = {}


def _haar_matrix():
    s = np.float32(1.0 / np.sqrt(2.0))
    A = np.zeros((N, N), dtype=np.float32)
    for i in range(N // 2):
        A[i, 2 * i] = s
        A[i, 2 * i + 1] = s
        A[64 + i, 2 * i] = -s
        A[64 + i, 2 * i + 1] = s
    return A


def _reference_numpy(inputs, A):
    # Fallback only: exact reference math on host (used if A is not Haar).
    x = np.einsum("ij,bpjqc->bpiqc", A, inputs)
    x = np.einsum("ij,bjpqc->bipqc", A, x)
    x = np.einsum("ij,bpqjc->bpqic", A, x)
    m = x.shape[1] // 2
    subs = [
        x[:, :m, :m, :m, :], x[:, :m, :m, m:, :],
        x[:, :m, m:, :m, :], x[:, :m, m:, m:, :],
        x[:, m:, :m, :m, :], x[:, m:, :m, m:, :],
        x[:, m:, m:, :m, :], x[:, m:, m:, m:, :],
    ]
    return np.concatenate(subs, axis=-1).astype(np.float32)


def _build_bass():
    import concourse.bacc as bacc
    import concourse.mybir as mybir
    import concourse.tile as tile

    f32 = mybir.dt.float32
    bf16 = mybir.dt.bfloat16

    # Bacc (not raw Bass): its compile() pipeline splits multi-sem waits into
    # EventSemaphore instructions — TRN2 instructions have one wait slot.
    nc = bacc.Bacc("TRN2", target_bir_lowering=False, debug=False)
    # x is host-pre-arranged to [d2, d1, t, m, c] (d3 = 2m + t) so the d3
    # butterfly reads/writes contiguous 512-elem runs per d1 slice.
    x = nc.dram_tensor("x", [N, SLAB, 2, 64, C], bf16, kind="ExternalInput")
    atf = nc.dram_tensor("atf", [N, N], bf16, kind="ExternalInput")
    atn = nc.dram_tensor("atn", [N, N], bf16, kind="ExternalInput")
    # y dims: (i2, o1, s1, s3, m*c) with i2 = s2*64 + o2 on the partition
    # axis. A chunk store is one 8 KiB contiguous run per partition.
    y = nc.dram_tensor("y", [N, 32, 2, 2, 64 * C], bf16, kind="ExternalOutput")

    with tile.TileContext(nc) as tc:
        with (
            tc.tile_pool(name="const", bufs=1) as cpool,
            tc.tile_pool(name="io", bufs=3) as tpool,
            tc.tile_pool(name="mid", bufs=3) as mpool,
            tc.tile_pool(name="out", bufs=3) as opool,
            tc.tile_pool(name="psum", bufs=2, space="PSUM") as ppool,
        ):
            atf_sb = cpool.tile([N, N], bf16)
            atn_sb = cpool.tile([N, N], bf16)

            for ki, (st, cnt) in enumerate(CHUNKS):
                # 1. load chunk: one DMA, 128 runs of cnt KiB.
                T = tpool.tile([N, cnt, 2, 64 * C], bf16, tag="T")
                nc.sync.dma_start(
                    out=T[:],
                    in_=x[:, st:st + cnt].rearrange("p a t m c -> p a t (m c)"),
                )
                if ki == 0:
                    # weights after the first bulk load is issued so the data
                    # pipeline starts immediately
                    nc.sync.dma_start(out=atf_sb[:], in_=atf[:, :])
                    nc.sync.dma_start(out=atn_sb[:], in_=atn[:, :])

                # Part of each chunk skips the DVE d3 butterfly; PE folds
                # it into PSUM accumulation instead (PE has slack). 4-wide
                # chunks fold entirely, shortening the DVE fill/drain chain.
                # The final chunk stays on DVE (idle by then): its post-DVE
                # chain is 4 matmuls instead of 16, shortening the tail.
                if ki == len(CHUNKS) - 1:
                    pe_cnt = 0
                else:
                    pe_cnt = 4 if cnt <= 4 else (4 if cnt == 12 else 0)
                dv_cnt = cnt - pe_cnt

                # 2. d3 butterfly (contiguous): W[:,:,0] = even+odd, [:,:,1] = odd-even
                # 3. d1 butterfly: U[:, 0, g] = W(2g)+W(2g+1), U[:, 1, g] = diff
                # (for the PE set, the same butterfly runs on raw T slices)
                U = mpool.tile([N, 2, cnt // 2, 512], bf16, tag="U")
                gd = dv_cnt // 2
                if dv_cnt:
                    W = mpool.tile([N, dv_cnt, 2, 64 * C], bf16, tag="W")
                    nc.vector.tensor_add(
                        out=W[:, :, 0], in0=T[:, :dv_cnt, 0], in1=T[:, :dv_cnt, 1]
                    )
                    nc.vector.tensor_sub(
                        out=W[:, :, 1], in0=T[:, :dv_cnt, 1], in1=T[:, :dv_cnt, 0]
                    )
                    Wp = W[:].rearrange("p (g u) t f -> p g u (t f)", u=2)
                    nc.vector.tensor_add(
                        out=U[:, 0, :gd], in0=Wp[:, :, 0], in1=Wp[:, :, 1]
                    )
                    nc.vector.tensor_sub(
                        out=U[:, 1, :gd], in0=Wp[:, :, 1], in1=Wp[:, :, 0]
                    )
                if pe_cnt:
                    Tp = T[:, dv_cnt:].rearrange("p (g u) t f -> p g u (t f)", u=2)
                    nc.vector.tensor_add(
                        out=U[:, 0, gd:], in0=Tp[:, :, 0], in1=Tp[:, :, 1]
                    )
                    nc.vector.tensor_sub(
                        out=U[:, 1, gd:], in0=Tp[:, :, 1], in1=Tp[:, :, 0]
                    )

                # staging: (o1_loc, s1, s3*m*c) — matches the y layout
                Yst = opool.tile([N, cnt // 2, 2, 512], bf16, tag="Yst")

                n_o1 = cnt // 2
                for q in range((n_o1 + 1) // 2):
                    # 4. d2 transform. DVE-set banks: one 512-row matmul per
                    # (o1, s1) slice (rhs already d3-butterflied). PE-set
                    # banks: the d3 butterfly rides the PSUM accumulation as
                    # two 256-row passes per half with +-0.5*A^T stationary.
                    jj = min(2, n_o1 - 2 * q)  # o1 slices in this PSUM group
                    ps = ppool.tile([N, jj, 2, 512], f32, tag="ps")
                    if 2 * q >= gd:
                        Uv = U[:].rearrange("p s g (u f) -> p s g u f", u=2)
                        # atp passes first, atn passes last: 2 ldweights/group
                        for j in range(jj):
                            for s1 in range(2):
                                g = 2 * q + j
                                bank = ps[:, j, s1].rearrange(
                                    "p (k f) -> p k f", k=2
                                )
                                nc.tensor.matmul(
                                    bank[:, 0], lhsT=atf_sb[:],
                                    rhs=Uv[:, s1, g, 0],
                                    start=True, stop=False,
                                )
                                nc.tensor.matmul(
                                    bank[:, 0], lhsT=atf_sb[:],
                                    rhs=Uv[:, s1, g, 1],
                                    start=False, stop=True,
                                )
                                nc.tensor.matmul(
                                    bank[:, 1], lhsT=atf_sb[:],
                                    rhs=Uv[:, s1, g, 1],
                                    start=True, stop=False,
                                )
                        for j in range(jj):
                            for s1 in range(2):
                                g = 2 * q + j
                                bank = ps[:, j, s1].rearrange(
                                    "p (k f) -> p k f", k=2
                                )
                                nc.tensor.matmul(
                                    bank[:, 1], lhsT=atn_sb[:],
                                    rhs=Uv[:, s1, g, 0],
                                    start=False, stop=True,
                                )
                    else:
                        for j in range(jj):
                            for s1 in range(2):
                                nc.tensor.matmul(
                                    ps[:, j, s1], lhsT=atf_sb[:],
                                    rhs=U[:, s1, 2 * q + j],
                                    start=True, stop=True,
                                )
                    # 5. evacuate: fully-contiguous 2048-elem copy f32->bf16
                    # (ACT; measured faster than a DVE CAST and its queue is
                    # already clear when the final chunk's matmuls finish)
                    nc.scalar.copy(Yst[:, 2 * q:2 * q + jj], ps[:])

                # 6. one store per chunk on SWDGE (gpsimd): stores never
                # head-of-line-block the load queue, and chunk-sized stores
                # keep 8-12 KiB contiguous runs per partition (DMA engines
                # lose ~25% per-packet efficiency at 4 KiB).
                o1s = st // 2
                nc.gpsimd.dma_start(
                    out=y[:, o1s:o1s + cnt // 2].rearrange(
                        "p q a k f -> p (q a k f)"
                    ),
                    in_=Yst[:].rearrange("p q a f -> p (q a f)"),
                )

    # All matmuls share one stationary matrix, but tile legalization emits an
    # InstLdweights per matmul (~130 ns of PE each). The PE array retains the
    # stationary between matmuls, so drop redundant loads, keeping one per
    # 4-matmul group: Bacc's move_matmul_waits_to_ldweights later merges a
    # matmul's extra waits onto the most recent ldweights, and per-group
    # retention keeps that merge target in its original program position.
    for blk in nc.main_func.blocks:
        keep = []
        mm_since_kept = 0
        last_key = None
        for i in blk.instructions:
            if isinstance(i, mybir.InstMatmult):
                mm_since_kept += 1
            elif isinstance(i, mybir.InstLdweights):
                si = i.sync_info
                clean = not si or (len(si.on_wait) == 0 and len(si.on_update) == 0)
                key = (i.ins[0].memref, i.ins[0].offset)
                if clean and key == last_key and mm_since_kept < 4:
                    continue
                last_key = key
                mm_since_kept = 0
            keep.append(i)
        blk.instructions[:] = keep

    nc.compile()
    return nc


def _make_in_maps(x, A):
    import ml_dtypes

    atf = np.ascontiguousarray(0.5 * A.T).astype(ml_dtypes.bfloat16)
    atn = np.ascontiguousarray(-0.5 * A.T).astype(ml_dtypes.bfloat16)
    in_maps = []
    for k in range(N_CORES):
        b, h = divmod(k, 2)
        # pre-arrange slab to [d2, d1, t, m, c] (d3 de-interleaved)
        xs = (
            x[b, h * SLAB:(h + 1) * SLAB]
            .reshape(SLAB, N, 64, 2, C)
            .transpose(1, 0, 3, 2, 4)
        )
        in_maps.append(
            {
                "x": np.ascontiguousarray(xs).astype(ml_dtypes.bfloat16),
                "atf": atf,
                "atn": atn,
            }
        )
    return in_maps


def kernel(**inputs):
    x = np.asarray(inputs["inputs"], dtype=np.float32)
    A = np.asarray(inputs["A"], dtype=np.float32)
    assert x.shape == (B, N, N, N, C), x.shape

    if not np.allclose(A, _haar_matrix(), atol=1e-5):
        # Kernel hardcodes the 2-tap Haar structure; fall back for generic A.
        return _reference_numpy(x, A)

    from concourse.bass_utils import run_bass_kernel_spmd

    if "nc" not in _BASS_CACHE:
        _BASS_CACHE["nc"] = _build_bass()
    nc = _BASS_CACHE["nc"]

    res = run_bass_kernel_spmd(
        nc, _make_in_maps(x, A), core_ids=list(range(N_CORES))
    )

    out = np.empty((B, 64, 64, 64, 8 * C), np.float32)
    for k in range(N_CORES):
        b, h = divmod(k, 2)
        # y: [i2, o1, s1, s3, m, c] with i2 = s2*64 + o2, o1 local to slab
        arr = np.asarray(res.results[k]["y"], dtype=np.float32).reshape(
            2, 64, 32, 2, 2, 64, C
        )
        # (s2, o2, o1, s1, s3, m, c) -> (o1, o2, m, s1, s2, s3, c)
        out[b, 32 * h:32 * h + 32] = (
            arr.transpose(2, 1, 5, 3, 0, 4, 6).reshape(32, 64, 64, 8 * C)
        )
    return out


# revision 19
# speedup vs baseline: 1.0093x; 1.0016x over previous
"""3D Haar DWT (clean-mode subband stack) on 8 Trainium2 NeuronCores, bf16 I/O.

Problem (hardcoded): inputs (4, 128, 128, 128, 4) f32, A (128, 128) f32 Haar
analysis operator. Output (4, 64, 64, 64, 32) f32 = 8 subbands stacked on the
channel axis (LLL, LLH, LHL, LHH, HLL, HLH, HHL, HHH) x 4 channels.

Sharding: pure data parallel over (batch, d1-half): core k handles
b = k // 2, d1 range [64*(k%2), 64*(k%2)+64). The Haar transform is a 2-tap
non-overlapping filter (rows of A touch only columns 2i, 2i+1), so splitting
d1 on an even boundary requires no communication.

The rel-err gate is 2e-2; a full bf16 pipeline measures ~6.5e-3, so all
device I/O is bf16 — 16 MiB/core of HBM traffic instead of 32, putting the
DMA roofline at ~47 us instead of ~94.

Per-core pipeline (slab host-converted to bf16 [d2, d1, d3par, d3pair, c],
i.e. even/odd d3 de-interleaved so every engine AP is contiguous):
  1. DMA in 1 MiB chunks (8 d1 slices), partitions = d2, 8 KiB runs.
     First/last 8 d1 are split 4+4 to shorten pipeline fill/drain.
  2. d3 butterfly on DVE (2 ops/chunk, all-contiguous bf16, 2x mode).
  3. d1 butterfly: sub on DVE; add offloaded to GPSIMD on the big middle
     chunks (it has slack after the store consolidation).
  4. d2 transform as one PE matmul per (s1, o1) slice with a single
     stationary weight matrix bf16(0.5*A^T) — both s2 halves come out on
     the PSUM partition axis, so each input column streams through PE once.
  5. PSUM -> SBUF evacuation on ACT: a single fully-contiguous 2048-elem
     copy/convert per 4 PSUM banks.
  6. One SWDGE store per chunk; y laid out [i2, o1, s1, s3, mc] so each
     store is one 8 KiB contiguous run per partition.

Scale bookkeeping: reference applies A (entries +-s, s=1/sqrt(2)) once per
axis: total s^3 per path. Here the d3/d1 butterflies apply +-1 and the
matmul applies 0.5*A (one s), so each path gets 0.5*s = s^3 exactly.
"""

import sys

import numpy as np

if "/opt/trn_rl_repo" not in sys.path:
    sys.path.insert(0, "/opt/trn_rl_repo")

B, N, C = 4, 128, 4
N_CORES = 8
SLAB = 64          # d1 extent per core
# (d1_start, d1_count) chunks: small ends shorten pipeline fill/drain
CHUNKS = (
    [(0, 4), (4, 4)]
    + [(8 + 12 * i, 12) for i in range(4)]
    + [(56, 4), (60, 4)]
)
GPSIMD_D1ADD = False  # gpsimd tensor ops are 4x slower and sat on the
                      # critical path (v2 measured +5.5 us); keep it DMA-only

_BASS_CACHE = {}


def _haar_matrix():
    s = np.float32(1.0 / np.sqrt(2.0))
    A = np.zeros((N, N), dtype=np.float32)
    for i in range(N // 2):
        A[i, 2 * i] = s
        A[i, 2 * i + 1] = s
        A[64 + i, 2 * i] = -s
        A[64 + i, 2 * i + 1] = s
    return A


def _reference_numpy(inputs, A):
    # Fallback only: exact reference math on host (used if A is not Haar).
    x = np.einsum("ij,bpjqc->bpiqc", A, inputs)
    x = np.einsum("ij,bjpqc->bipqc", A, x)
    x = np.einsum("ij,bpqjc->bpqic", A, x)
    m = x.shape[1] // 2
    subs = [
        x[:, :m, :m, :m, :], x[:, :m, :m, m:, :],
        x[:, :m, m:, :m, :], x[:, :m, m:, m:, :],
        x[:, m:, :m, :m, :], x[:, m:, :m, m:, :],
        x[:, m:, m:, :m, :], x[:, m:, m:, m:, :],
    ]
    return np.concatenate(subs, axis=-1).astype(np.float32)


def _build_bass():
    import concourse.bacc as bacc
    import concourse.mybir as mybir
    import concourse.tile as tile

    f32 = mybir.dt.float32
    bf16 = mybir.dt.bfloat16

    # Bacc (not raw Bass): its compile() pipeline splits multi-sem waits into
    # EventSemaphore instructions — TRN2 instructions have one wait slot.
    nc = bacc.Bacc("TRN2", target_bir_lowering=False, debug=False)
    # x is host-pre-arranged to [d2, d1, t, m, c] (d3 = 2m + t) so the d3
    # butterfly reads/writes contiguous 512-elem runs per d1 slice.
    x = nc.dram_tensor("x", [N, SLAB, 2, 64, C], bf16, kind="ExternalInput")
    atf = nc.dram_tensor("atf", [N, N], bf16, kind="ExternalInput")
    atn = nc.dram_tensor("atn", [N, N], bf16, kind="ExternalInput")
    # y dims: (i2, o1, s1, s3, m*c) with i2 = s2*64 + o2 on the partition
    # axis. A chunk store is one 8 KiB contiguous run per partition.
    y = nc.dram_tensor("y", [N, 32, 2, 2, 64 * C], bf16, kind="ExternalOutput")

    with tile.TileContext(nc) as tc:
        with (
            tc.tile_pool(name="const", bufs=1) as cpool,
            tc.tile_pool(name="io", bufs=3) as tpool,
            tc.tile_pool(name="mid", bufs=3) as mpool,
            tc.tile_pool(name="out", bufs=3) as opool,
            tc.tile_pool(name="psum", bufs=2, space="PSUM") as ppool,
        ):
            atf_sb = cpool.tile([N, N], bf16)
            atn_sb = cpool.tile([N, N], bf16)

            for ki, (st, cnt) in enumerate(CHUNKS):
                # 1. load chunk: one DMA, 128 runs of cnt KiB.
                T = tpool.tile([N, cnt, 2, 64 * C], bf16, tag="T")
                nc.sync.dma_start(
                    out=T[:],
                    in_=x[:, st:st + cnt].rearrange("p a t m c -> p a t (m c)"),
                )
                if ki == 0:
                    # weights after the first bulk load is issued so the data
                    # pipeline starts immediately
                    nc.sync.dma_start(out=atf_sb[:], in_=atf[:, :])
                    nc.sync.dma_start(out=atn_sb[:], in_=atn[:, :])

                # Part of each chunk skips the DVE d3 butterfly; PE folds
                # it into PSUM accumulation instead (PE has slack). 4-wide
                # chunks fold entirely, shortening the DVE fill/drain chain.
                # The final chunk stays on DVE (idle by then): its post-DVE
                # chain is 4 matmuls instead of 16, shortening the tail.
                if ki == len(CHUNKS) - 1:
                    pe_cnt = 0
                else:
                    pe_cnt = 4 if cnt <= 4 else (4 if cnt == 12 else 0)
                dv_cnt = cnt - pe_cnt

                # 2. d3 butterfly (contiguous): W[:,:,0] = even+odd, [:,:,1] = odd-even
                # 3. d1 butterfly: U[:, 0, g] = W(2g)+W(2g+1), U[:, 1, g] = diff
                # (for the PE set, the same butterfly runs on raw T slices)
                U = mpool.tile([N, 2, cnt // 2, 512], bf16, tag="U")
                gd = dv_cnt // 2
                if dv_cnt:
                    W = mpool.tile([N, dv_cnt, 2, 64 * C], bf16, tag="W")
                    nc.vector.tensor_add(
                        out=W[:, :, 0], in0=T[:, :dv_cnt, 0], in1=T[:, :dv_cnt, 1]
                    )
                    nc.vector.tensor_sub(
                        out=W[:, :, 1], in0=T[:, :dv_cnt, 1], in1=T[:, :dv_cnt, 0]
                    )
                    Wp = W[:].rearrange("p (g u) t f -> p g u (t f)", u=2)
                    nc.vector.tensor_add(
                        out=U[:, 0, :gd], in0=Wp[:, :, 0], in1=Wp[:, :, 1]
                    )
                    nc.vector.tensor_sub(
                        out=U[:, 1, :gd], in0=Wp[:, :, 1], in1=Wp[:, :, 0]
                    )
                if pe_cnt:
                    Tp = T[:, dv_cnt:].rearrange("p (g u) t f -> p g u (t f)", u=2)
                    nc.vector.tensor_add(
                        out=U[:, 0, gd:], in0=Tp[:, :, 0], in1=Tp[:, :, 1]
                    )
                    nc.vector.tensor_sub(
                        out=U[:, 1, gd:], in0=Tp[:, :, 1], in1=Tp[:, :, 0]
                    )

                # staging: (o1_loc, s1, s3*m*c) — matches the y layout
                Yst = opool.tile([N, cnt // 2, 2, 512], bf16, tag="Yst")

                n_o1 = cnt // 2
                for q in range((n_o1 + 1) // 2):
                    # 4. d2 transform. DVE-set banks: one 512-row matmul per
                    # (o1, s1) slice (rhs already d3-butterflied). PE-set
                    # banks: the d3 butterfly rides the PSUM accumulation as
                    # two 256-row passes per half with +-0.5*A^T stationary.
                    jj = min(2, n_o1 - 2 * q)  # o1 slices in this PSUM group
                    ps = ppool.tile([N, jj, 2, 512], f32, tag="ps")
                    if 2 * q >= gd:
                        Uv = U[:].rearrange("p s g (u f) -> p s g u f", u=2)
                        # atp passes first, atn passes last: 2 ldweights/group
                        for j in range(jj):
                            for s1 in range(2):
                                g = 2 * q + j
                                bank = ps[:, j, s1].rearrange(
                                    "p (k f) -> p k f", k=2
                                )
                                nc.tensor.matmul(
                                    bank[:, 0], lhsT=atf_sb[:],
                                    rhs=Uv[:, s1, g, 0],
                                    start=True, stop=False,
                                )
                                nc.tensor.matmul(
                                    bank[:, 0], lhsT=atf_sb[:],
                                    rhs=Uv[:, s1, g, 1],
                                    start=False, stop=True,
                                )
                                nc.tensor.matmul(
                                    bank[:, 1], lhsT=atf_sb[:],
                                    rhs=Uv[:, s1, g, 1],
                                    start=True, stop=False,
                                )
                        for j in range(jj):
                            for s1 in range(2):
                                g = 2 * q + j
                                bank = ps[:, j, s1].rearrange(
                                    "p (k f) -> p k f", k=2
                                )
                                nc.tensor.matmul(
                                    bank[:, 1], lhsT=atn_sb[:],
                                    rhs=Uv[:, s1, g, 0],
                                    start=False, stop=True,
                                )
                    else:
                        for j in range(jj):
                            for s1 in range(2):
                                nc.tensor.matmul(
                                    ps[:, j, s1], lhsT=atf_sb[:],
                                    rhs=U[:, s1, 2 * q + j],
                                    start=True, stop=True,
                                )
                    # 5. evacuate: fully-contiguous 2048-elem copy f32->bf16
                    # (ACT; measured faster than a DVE CAST and its queue is
                    # already clear when the final chunk's matmuls finish)
                    nc.scalar.copy(Yst[:, 2 * q:2 * q + jj], ps[:])

                # 6. one store per chunk on SWDGE (gpsimd): stores never
                # head-of-line-block the load queue, and chunk-sized stores
                # keep 8-12 KiB contiguous runs per partition (DMA engines
                # lose ~25% per-packet efficiency at 4 KiB). The last two
                # stores would serialize on the SWDGE ring right at the tail,
                # so the second-to-last rides the (idle by then) HWDGE ring.
                o1s = st // 2
                store_eng = nc.sync if ki == len(CHUNKS) - 2 else nc.gpsimd
                store_eng.dma_start(
                    out=y[:, o1s:o1s + cnt // 2].rearrange(
                        "p q a k f -> p (q a k f)"
                    ),
                    in_=Yst[:].rearrange("p q a f -> p (q a f)"),
                )

    # All matmuls share one stationary matrix, but tile legalization emits an
    # InstLdweights per matmul (~130 ns of PE each). The PE array retains the
    # stationary between matmuls, so drop redundant loads, keeping one per
    # 4-matmul group: Bacc's move_matmul_waits_to_ldweights later merges a
    # matmul's extra waits onto the most recent ldweights, and per-group
    # retention keeps that merge target in its original program position.
    for blk in nc.main_func.blocks:
        keep = []
        mm_since_kept = 0
        last_key = None
        for i in blk.instructions:
            if isinstance(i, mybir.InstMatmult):
                mm_since_kept += 1
            elif isinstance(i, mybir.InstLdweights):
                si = i.sync_info
                clean = not si or (len(si.on_wait) == 0 and len(si.on_update) == 0)
                key = (i.ins[0].memref, i.ins[0].offset)
                if clean and key == last_key and mm_since_kept < 4:
                    continue
                last_key = key
                mm_since_kept = 0
            keep.append(i)
        blk.instructions[:] = keep

    nc.compile()
    return nc


def _make_in_maps(x, A):
    import ml_dtypes

    atf = np.ascontiguousarray(0.5 * A.T).astype(ml_dtypes.bfloat16)
    atn = np.ascontiguousarray(-0.5 * A.T).astype(ml_dtypes.bfloat16)
    in_maps = []
    for k in range(N_CORES):
        b, h = divmod(k, 2)
        # pre-arrange slab to [d2, d1, t, m, c] (d3 de-interleaved)
        xs = (
            x[b, h * SLAB:(h + 1) * SLAB]
            .reshape(SLAB, N, 64, 2, C)
            .transpose(1, 0, 3, 2, 4)
        )
        in_maps.append(
            {
                "x": np.ascontiguousarray(xs).astype(ml_dtypes.bfloat16),
                "atf": atf,
                "atn": atn,
            }
        )
    return in_maps


def kernel(**inputs):
    x = np.asarray(inputs["inputs"], dtype=np.float32)
    A = np.asarray(inputs["A"], dtype=np.float32)
    assert x.shape == (B, N, N, N, C), x.shape

    if not np.allclose(A, _haar_matrix(), atol=1e-5):
        # Kernel hardcodes the 2-tap Haar structure; fall back for generic A.
        return _reference_numpy(x, A)

    from concourse.bass_utils import run_bass_kernel_spmd

    if "nc" not in _BASS_CACHE:
        _BASS_CACHE["nc"] = _build_bass()
    nc = _BASS_CACHE["nc"]

    res = run_bass_kernel_spmd(
        nc, _make_in_maps(x, A), core_ids=list(range(N_CORES))
    )

    out = np.empty((B, 64, 64, 64, 8 * C), np.float32)
    for k in range(N_CORES):
        b, h = divmod(k, 2)
        # y: [i2, o1, s1, s3, m, c] with i2 = s2*64 + o2, o1 local to slab
        arr = np.asarray(res.results[k]["y"], dtype=np.float32).reshape(
            2, 64, 32, 2, 2, 64, C
        )
        # (s2, o2, o1, s1, s3, m, c) -> (o1, o2, m, s1, s2, s3, c)
        out[b, 32 * h:32 * h + 32] = (
            arr.transpose(2, 1, 5, 3, 0, 4, 6).reshape(32, 64, 64, 8 * C)
        )
    return out
